# revision 1
# baseline (speedup 1.0000x reference)
"""Causal GQA self-attention on 8 Trainium2 NeuronCores.

Problem: B=2, S=2048, HIDDEN=2048, 16 q-heads, 4 kv-heads, head_dim=128, fp32.

Sharding: core c = 4*b + g  (b = batch, g = head-group).
Each core owns batch b and q-heads [4g, 4g+4) plus their shared kv-head g.

Per-core pipeline (f16 operands, fp32 PSUM accumulation everywhere):
  A. Projections, per 512-col chunk of X.T: QT[d,h,s], KT[d,s] via
     weight-stationary matmuls; V[s,d] directly (xt-stationary, wv moving)
     with bias folded in as a K=1 ones-row matmul.
  B. Attention per (chunk, head): ST = KT_j.T @ QT -> P = exp (ACT, fused
     scale + key-mask bias) -> causal triangle mul on diagonal tiles (DVE)
     -> attnT += V_j.T @ P (PE); rowsum accumulated on DVE (acc += P), one
     final ones.T @ acc matmul; 1/l via reciprocal_approx_fast; normalize.
  C. AllGather attnT f16 across the 4 cores of the batch; O-projection with
     this core's 512 Wo columns.  O-proj for chunk c is emitted after
     attention of chunk c+1 so the PE never idles waiting on the collective;
     the last chunk AllGathers per-head with O-proj accumulating per arrival
     to shrink the tail.
Host gathers: out[b][:, 512g:512(g+1)] = core(b,g) outT.T.
"""

import numpy as np

HID = 2048
S = 2048
B = 2
NH = 16          # q heads total
D = 128          # head dim
G = 4            # head groups == cores per batch
HPG = NH // G    # q heads per group (4)
CH = 512         # seq chunk (free dim of moving operands)
NCH = S // CH    # 4 chunks
NKT = S // 128   # 16 key tiles
SCALE = 1.0 / float(np.sqrt(D))

_CACHED_NC = None


def _build_nc(reps=1):
    import concourse.mybir as mybir
    import concourse.tile as tile
    from concourse import bacc

    F32 = mybir.dt.float32
    F16 = mybir.dt.float16
    Copy = mybir.ActivationFunctionType.Copy
    Exp = mybir.ActivationFunctionType.Exp

    nc = bacc.Bacc("TRN2", target_bir_lowering=False, debug=False,
                   num_devices=8)

    # ---- per-core input shards (f16 unless noted) ----
    xt = nc.declare_dram_parameter("xt", [HID, S], F16, isOutput=False)
    wq = nc.declare_dram_parameter("wq", [HID, HPG * D], F16, isOutput=False)
    wk = nc.declare_dram_parameter("wk", [HID, D], F16, isOutput=False)
    wv = nc.declare_dram_parameter("wv", [HID, D], F16, isOutput=False)
    wo = nc.declare_dram_parameter("wo", [HID, CH], F16, isOutput=False)
    bq = nc.declare_dram_parameter("bq", [D, HPG], F32, isOutput=False)
    bk = nc.declare_dram_parameter("bk", [D, 1], F32, isOutput=False)
    bvr = nc.declare_dram_parameter("bvr", [1, HPG * D], F16, isOutput=False)
    bo = nc.declare_dram_parameter("bo", [D, HPG], F32, isOutput=False)
    mask = nc.declare_dram_parameter("mask", [128, 128], F16, isOutput=False)
    keybias = nc.declare_dram_parameter("keybias", [128, NKT], F32, isOutput=False)
    ones = nc.declare_dram_parameter("ones", [128, 1], F16, isOutput=False)
    onesrow = nc.declare_dram_parameter("onesrow", [1, 128], F16, isOutput=False)
    out = nc.declare_dram_parameter("out", [CH, S], F16, isOutput=True)

    groups = [[0, 1, 2, 3], [4, 5, 6, 7]]

    with tile.TileContext(nc) as tc:
        for _rep in range(reps):
            with (
                tc.tile_pool(name="persist", bufs=1) as persist,
                tc.tile_pool(name="dram", bufs=3, space="DRAM") as dram,
            ):
                # ---- persistent SBUF state ----
                qt_sb = persist.tile([128, HPG, S], F16)      # QT [d, h, s]
                kt_sb = persist.tile([128, S], F16)           # KT [d, s]
                v_sb = persist.tile([128, NKT, D], F16)       # V  [s, j, d]
                wq_sb = persist.tile([128, NKT, HPG * D], F16)
                wk_sb = persist.tile([128, NKT, D], F16)
                wv_sb = persist.tile([128, NKT, D], F16)
                wo_sb = persist.tile([128, NKT, CH], F16)
                mask_sb = persist.tile([128, 128], F16)
                kb_sb = persist.tile([128, NKT], F32)
                ones_sb = persist.tile([128, 1], F16)
                or_sb = persist.tile([1, 128], F16)
                bvr_sb = persist.tile([1, HPG * D], F16)
                bq_sb = persist.tile([D, HPG], F32)
                bk_sb = persist.tile([D, 1], F32)
                bo_sb = persist.tile([D, HPG], F32)

                # weights first (phase A needs all of wq/wk/wv for chunk 0)
                wq_r = wq.ap().rearrange("(t p) n -> p t n", p=128)
                nc.gpsimd.dma_start(out=wq_sb[:, :8, :], in_=wq_r[:, :8, :])
                nc.sync.dma_start(
                    out=wk_sb, in_=wk.ap().rearrange("(t p) n -> p t n", p=128))
                nc.sync.dma_start(
                    out=wv_sb, in_=wv.ap().rearrange("(t p) n -> p t n", p=128))
                nc.gpsimd.dma_start(out=wq_sb[:, 8:, :], in_=wq_r[:, 8:, :])
                nc.scalar.dma_start(out=or_sb, in_=onesrow.ap())
                nc.scalar.dma_start(out=bvr_sb, in_=bvr.ap())
                nc.sync.dma_start(out=bq_sb, in_=bq.ap())
                nc.sync.dma_start(out=bk_sb, in_=bk.ap())
                # phase-B constants (not needed for a while)
                nc.scalar.dma_start(out=mask_sb, in_=mask.ap())
                nc.scalar.dma_start(out=kb_sb, in_=keybias.ap())
                nc.scalar.dma_start(out=ones_sb, in_=ones.ap())
                nc.scalar.dma_start(out=bo_sb, in_=bo.ap())

                # ================= Phase A: projections =================
                with (
                    tc.tile_pool(name="xs", bufs=6) as xs,
                    tc.tile_pool(name="psq", bufs=5, space="PSUM") as psq,
                    tc.tile_pool(name="psk", bufs=2, space="PSUM") as psk,
                    tc.tile_pool(name="psv", bufs=1, space="PSUM") as psv,
                ):
                    for c in range(NCH):
                        sq = slice(c * CH, (c + 1) * CH)
                        ps_q = [psq.tile([128, CH], F32, name="ps_q")
                                for _ in range(HPG)]
                        ps_k = psk.tile([128, CH], F32, name="ps_k")
                        ps_v = psv.tile([128, HPG * D], F32, name="ps_v")
                        # single full-bank start: init V bank to the (tiled)
                        # bias -- PSUM start resets the whole bank, so the
                        # per-quarter matmuls below must all accumulate
                        nc.tensor.matmul(ps_v, lhsT=or_sb, rhs=bvr_sb,
                                         start=True, stop=False)
                        for t in range(NKT):
                            xt_t = xs.tile([128, CH], F16, name="xt_t")
                            nc.gpsimd.dma_start(
                                out=xt_t, in_=xt[t * 128:(t + 1) * 128, sq])
                            st, sp = (t == 0), (t == NKT - 1)
                            for h in range(HPG):
                                nc.tensor.matmul(
                                    ps_q[h],
                                    lhsT=wq_sb[:, t, h * D:(h + 1) * D],
                                    rhs=xt_t, start=st, stop=sp)
                            nc.tensor.matmul(ps_k, lhsT=wk_sb[:, t, :], rhs=xt_t,
                                             start=st, stop=sp)
                            # V[s, d] directly: xt slices stationary, wv moving
                            for u in range(HPG):
                                nc.tensor.matmul(
                                    ps_v[:, u * D:(u + 1) * D],
                                    lhsT=xt_t[:, u * 128:(u + 1) * 128],
                                    rhs=wv_sb[:, t, :], start=False, stop=sp)
                        for h in range(HPG):
                            nc.vector.tensor_scalar_add(qt_sb[:, h, sq],
                                                        ps_q[h],
                                                        bq_sb[:, h:h + 1])
                        nc.vector.tensor_scalar_add(kt_sb[:, sq], ps_k, bk_sb)
                        for u in range(HPG):
                            nc.scalar.copy(v_sb[:, 4 * c + u, :],
                                           ps_v[:, u * D:(u + 1) * D])
                        if c == 0:
                            nc.gpsimd.dma_start(
                                out=wo_sb,
                                in_=wo.ap().rearrange("(t p) n -> p t n", p=128))

                # ================= Phase B: attention + O-projection =================
                with (
                    tc.tile_pool(name="ps_s", bufs=3, space="PSUM") as ps_s_pool,
                    tc.tile_pool(name="ps_pv", bufs=2, space="PSUM") as ps_pv_pool,
                    tc.tile_pool(name="ps_l", bufs=1, space="PSUM") as ps_l_pool,
                    tc.tile_pool(name="ps_o", bufs=2, space="PSUM") as ps_o_pool,
                    tc.tile_pool(name="pp", bufs=5) as pp,
                    tc.tile_pool(name="accp", bufs=2) as accp,
                    tc.tile_pool(name="att", bufs=2) as att,
                    tc.tile_pool(name="rbp", bufs=2) as rbp,
                    tc.tile_pool(name="mo", bufs=2) as mo,
                    tc.tile_pool(name="ost", bufs=3) as ost,
                ):
                    ag_ins = {}
                    ag_outs = {}

                    def attn_chunk(c):
                        sq = slice(c * CH, (c + 1) * CH)
                        njt = 4 * c + 4
                        ag_in = dram.tile([HPG, 128, CH], F16, name="ag_in")
                        ag_ins[c] = ag_in
                        for h in range(HPG):
                            ps_pv = ps_pv_pool.tile([128, CH], F32, name="ps_pv")
                            ps_l = ps_l_pool.tile([1, CH], F32, name="ps_l")
                            for j in range(njt):
                                r = max(0, j - 4 * c)
                                cs = slice(128 * r, CH)
                                qs_ = slice(c * CH + 128 * r, (c + 1) * CH)
                                ps_s = ps_s_pool.tile([128, CH], F32, name="ps_s")
                                nc.tensor.matmul(
                                    ps_s[:, cs],
                                    lhsT=kt_sb[:, j * 128:(j + 1) * 128],
                                    rhs=qt_sb[:, h, qs_], start=True, stop=True)
                                p_sb = pp.tile([128, CH], F16, name="p_sb")
                                nc.scalar.activation(p_sb[:, cs], ps_s[:, cs],
                                                     Exp, scale=SCALE,
                                                     bias=kb_sb[:, j:j + 1])
                                if j >= 4 * c:
                                    nc.vector.tensor_mul(
                                        p_sb[:, 128 * r:128 * (r + 1)],
                                        p_sb[:, 128 * r:128 * (r + 1)],
                                        mask_sb)
                                st, sp = (j == 0), (j == njt - 1)
                                nc.tensor.matmul(ps_pv[:, cs], lhsT=v_sb[:, j, :],
                                                 rhs=p_sb[:, cs],
                                                 start=st, stop=sp)
                                nc.tensor.matmul(ps_l[:, cs], lhsT=ones_sb,
                                                 rhs=p_sb[:, cs],
                                                 start=st, stop=sp)
                            rl = rbp.tile([1, CH], F32, name="rl")
                            nc.vector.reciprocal(rl, ps_l)
                            rb = rbp.tile([128, CH], F32, name="rb")
                            nc.gpsimd.partition_broadcast(rb, rl, channels=128)
                            at_sb = att.tile([128, CH], F16, name="at_sb")
                            nc.vector.tensor_mul(at_sb, ps_pv, rb)
                            nc.sync.dma_start(out=ag_ins[c][h], in_=at_sb)
                        ag_out = dram.tile([G, HPG, 128, CH], F16,
                                           name="ag_out")
                        nc.gpsimd.collective_compute(
                            "AllGather", mybir.AluOpType.bypass,
                            replica_groups=groups,
                            ins=[ag_ins[c].opt()], outs=[ag_out.opt()],
                        )
                        ag_outs[c] = ag_out

                    def oproj_chunk(c):
                        sq = slice(c * CH, (c + 1) * CH)
                        m_all = mo.tile([128, NKT, CH], F16, name="m_all")
                        ag_r = ag_outs[c].rearrange("g h p n -> p (g h) n")
                        for ct in range(NKT):
                            eng = nc.sync if ct % 2 == 0 else nc.scalar
                            eng.dma_start(out=m_all[:, ct, :], in_=ag_r[:, ct, :])
                        for t in range(HPG):
                            ps_o = ps_o_pool.tile([128, CH], F32, name="ps_o")
                            for ct in range(NKT):
                                nc.tensor.matmul(
                                    ps_o,
                                    lhsT=wo_sb[:, ct, t * 128:(t + 1) * 128],
                                    rhs=m_all[:, ct, :],
                                    start=(ct == 0), stop=(ct == NKT - 1))
                            o_sb = ost.tile([128, CH], F16, name="o_sb")
                            nc.vector.tensor_scalar_add(o_sb, ps_o,
                                                        bo_sb[:, t:t + 1])
                            nc.sync.dma_start(
                                out=out[t * 128:(t + 1) * 128, sq], in_=o_sb)

                    # Schedule: every AllGather hides under later PE work --
                    # AG0 under attn1+attn2, AG1 under attn2+oproj0, AG2 under
                    # oproj0+attn3+oproj1, AG3 under oproj1+oproj2.
                    attn_chunk(0)
                    attn_chunk(1)
                    attn_chunk(2)
                    oproj_chunk(0)
                    attn_chunk(3)
                    oproj_chunk(1)
                    oproj_chunk(2)
                    oproj_chunk(3)

    nc.compile()
    return nc


def _host_consts():
    p = np.arange(128)
    mask = (p[None, :] >= p[:, None]).astype(np.float16)  # col >= row
    ones = np.ones((128, 1), dtype=np.float16)
    onesrow = np.ones((1, 128), dtype=np.float16)
    return mask, ones, onesrow


def kernel(hidden_states, attention_mask, Wq, bq, Wk, bk, Wv, bv, Wo, bo):
    from concourse.bass_utils import run_bass_kernel_spmd

    global _CACHED_NC
    if _CACHED_NC is None:
        _CACHED_NC = _build_nc()
    nc = _CACHED_NC

    X = np.asarray(hidden_states, dtype=np.float32)
    am = np.asarray(attention_mask).astype(np.float32)  # [B, S] key mask
    Wq = np.asarray(Wq, np.float16)
    Wk = np.asarray(Wk, np.float16)
    Wv = np.asarray(Wv, np.float16)
    Wo = np.asarray(Wo, np.float16)
    mask, ones, onesrow = _host_consts()

    in_maps = []
    for c in range(8):
        b, g = divmod(c, G)
        qs = slice(g * HPG * D, (g + 1) * HPG * D)   # q-head cols of group g
        ks = slice(g * D, (g + 1) * D)               # kv-head cols of group g
        in_maps.append({
            "xt": np.ascontiguousarray(X[b].T.astype(np.float16)),
            "wq": np.ascontiguousarray(Wq[:, qs]),
            "wk": np.ascontiguousarray(Wk[:, ks]),
            "wv": np.ascontiguousarray(Wv[:, ks]),
            "wo": np.ascontiguousarray(Wo[:, qs]),   # hid cols [512g, 512g+512)
            "bq": np.ascontiguousarray(
                np.asarray(bq, np.float32)[qs].reshape(HPG, D).T),
            "bk": np.asarray(bk, np.float32)[ks].reshape(D, 1).copy(),
            "bvr": np.ascontiguousarray(
                np.tile(np.asarray(bv, np.float16)[ks], HPG).reshape(1, HPG * D)),
            "bo": np.ascontiguousarray(
                np.asarray(bo, np.float32)[qs].reshape(HPG, D).T),
            "mask": mask.copy(),
            "keybias": np.ascontiguousarray(
                ((1.0 - am[b]) * -10000.0).astype(np.float32)
                .reshape(NKT, 128).T),
            "ones": ones.copy(),
            "onesrow": onesrow.copy(),
        })

    global _last_in_maps
    _last_in_maps = in_maps
    res = run_bass_kernel_spmd(nc, in_maps, core_ids=list(range(8)))
    out = np.empty((B, S, HID), dtype=np.float32)
    for c in range(8):
        b, g = divmod(c, G)
        out[b][:, g * CH:(g + 1) * CH] = res.results[c]["out"].T.astype(np.float32)
    return out



# revision 3
# speedup vs baseline: 1.1112x; 1.1112x over previous
"""Causal GQA self-attention on 8 Trainium2 NeuronCores.

Problem: B=2, S=2048, HIDDEN=2048, 16 q-heads, 4 kv-heads, head_dim=128, fp32.

Sharding: core c = 4*b + g  (b = batch, g = head-group).
Each core owns batch b and q-heads [4g, 4g+4) plus their shared kv-head g.

Per-core pipeline (f16 operands, fp32 PSUM accumulation everywhere):
  A. Projections, per 512-col chunk of X.T: QT[d,h,s], KT[d,s] via
     weight-stationary matmuls; V[s,d] directly (xt-stationary, wv moving)
     with bias folded in as a K=1 ones-row matmul.  Weight tiles stream in
     per-t so the first matmul starts as soon as wq/wk/wv tile 0 lands.
  B. Attention per (chunk, head): ST = KT_j.T @ QT -> P = exp (ACT, fused
     scale + key-mask bias) -> causal triangle mul on diagonal tiles (DVE)
     -> attnT += V_j.T @ P (PE) and rowsum l += ones.T @ P (PE).  The
     scores matmul for tile j+1 is emitted ahead of PV(j) so the PE never
     waits on the ACT exp.  Normalize via reciprocal_approx_fast (DVE) +
     partition_broadcast (GPSIMD) + tensor mul.  AllGather attnT f16 per
     chunk across the 4 cores of the batch, fired right after the chunk's
     attention -- all 4 AllGathers complete while the PE is still busy
     with later attention chunks.
  C. After all attention: O-projection per chunk with this core's 512 Wo
     columns, reading the gathered heads from DRAM (loads spread across
     sync/vector/gpsimd queues, prefetched one chunk ahead).
Host gathers: out[b][:, 512g:512(g+1)] = core(b,g) outT.T.
"""

import numpy as np

HID = 2048
S = 2048
B = 2
NH = 16          # q heads total
D = 128          # head dim
G = 4            # head groups == cores per batch
HPG = NH // G    # q heads per group (4)
CH = 512         # seq chunk (free dim of moving operands)
NCH = S // CH    # 4 chunks
NKT = S // 128   # 16 key tiles
SCALE = 1.0 / float(np.sqrt(D))

_CACHED_NC = None


def _build_nc(reps=1):
    import concourse.mybir as mybir
    import concourse.tile as tile
    from concourse import bacc

    F32 = mybir.dt.float32
    F16 = mybir.dt.float16
    Exp = mybir.ActivationFunctionType.Exp

    nc = bacc.Bacc("TRN2", target_bir_lowering=False, debug=False,
                   num_devices=8)

    # ---- per-core input shards (f16 unless noted) ----
    xt = nc.declare_dram_parameter("xt", [HID, S], F16, isOutput=False)
    wq = nc.declare_dram_parameter("wq", [HID, HPG * D], F16, isOutput=False)
    wk = nc.declare_dram_parameter("wk", [HID, D], F16, isOutput=False)
    wv = nc.declare_dram_parameter("wv", [HID, D], F16, isOutput=False)
    wo = nc.declare_dram_parameter("wo", [HID, CH], F16, isOutput=False)
    bq = nc.declare_dram_parameter("bq", [D, HPG], F32, isOutput=False)
    bk = nc.declare_dram_parameter("bk", [D, 1], F32, isOutput=False)
    bvr = nc.declare_dram_parameter("bvr", [1, HPG * D], F16, isOutput=False)
    bo = nc.declare_dram_parameter("bo", [D, HPG], F32, isOutput=False)
    mask = nc.declare_dram_parameter("mask", [128, 128], F16, isOutput=False)
    keybias = nc.declare_dram_parameter("keybias", [128, NKT], F32, isOutput=False)
    ones = nc.declare_dram_parameter("ones", [128, 1], F16, isOutput=False)
    onesrow = nc.declare_dram_parameter("onesrow", [1, 128], F16, isOutput=False)
    out = nc.declare_dram_parameter("out", [CH, S], F16, isOutput=True)

    groups = [[0, 1, 2, 3], [4, 5, 6, 7]]

    with tile.TileContext(nc) as tc:
        for _rep in range(reps):
            with (
                tc.tile_pool(name="persist", bufs=1) as persist,
                tc.tile_pool(name="dram", bufs=8, space="DRAM") as dram,
            ):
                # ---- persistent SBUF state ----
                qt_sb = persist.tile([128, HPG, S], F16)      # QT [d, h, s]
                kt_sb = persist.tile([128, S], F16)           # KT [d, s]
                v_sb = persist.tile([128, NKT, D], F16)       # V  [s, j, d]
                wq_sb = persist.tile([128, NKT, HPG * D], F16)
                wk_sb = persist.tile([128, NKT, D], F16)
                wv_sb = persist.tile([128, NKT, D], F16)
                wo_sb = persist.tile([128, NKT, CH], F16)
                mask_sb = persist.tile([128, 128], F16)
                kb_sb = persist.tile([128, NKT], F32)
                ones_sb = persist.tile([128, 1], F16)
                or_sb = persist.tile([1, 128], F16)
                bvr_sb = persist.tile([1, HPG * D], F16)
                bq_sb = persist.tile([D, HPG], F32)
                bk_sb = persist.tile([D, 1], F32)
                bo_sb = persist.tile([D, HPG], F32)

                # weight tiles stream per-t so phase A's first matmuls only
                # wait on tile 0 of wq/wk/wv
                wq_r = wq.ap().rearrange("(t p) n -> p t n", p=128)
                wk_r = wk.ap().rearrange("(t p) n -> p t n", p=128)
                wv_r = wv.ap().rearrange("(t p) n -> p t n", p=128)
                for t in range(NKT):
                    nc.sync.dma_start(out=wq_sb[:, t, :], in_=wq_r[:, t, :])
                    nc.scalar.dma_start(out=wk_sb[:, t, :], in_=wk_r[:, t, :])
                    nc.scalar.dma_start(out=wv_sb[:, t, :], in_=wv_r[:, t, :])
                nc.scalar.dma_start(out=or_sb, in_=onesrow.ap())
                nc.scalar.dma_start(out=bvr_sb, in_=bvr.ap())
                nc.sync.dma_start(out=bq_sb, in_=bq.ap())
                nc.sync.dma_start(out=bk_sb, in_=bk.ap())
                # phase-B constants (not needed for a while)
                nc.sync.dma_start(out=mask_sb, in_=mask.ap())
                nc.sync.dma_start(out=kb_sb, in_=keybias.ap())
                nc.sync.dma_start(out=ones_sb, in_=ones.ap())
                nc.sync.dma_start(out=bo_sb, in_=bo.ap())

                # ================= Phase A: projections =================
                with (
                    tc.tile_pool(name="xs", bufs=6) as xs,
                    tc.tile_pool(name="psq", bufs=5, space="PSUM") as psq,
                    tc.tile_pool(name="psk", bufs=2, space="PSUM") as psk,
                    tc.tile_pool(name="psv", bufs=1, space="PSUM") as psv,
                ):
                    for c in range(NCH):
                        sq = slice(c * CH, (c + 1) * CH)
                        ps_q = [psq.tile([128, CH], F32, name="ps_q")
                                for _ in range(HPG)]
                        ps_k = psk.tile([128, CH], F32, name="ps_k")
                        ps_v = psv.tile([128, HPG * D], F32, name="ps_v")
                        # single full-bank start: init V bank to the (tiled)
                        # bias -- PSUM start resets the whole bank, so the
                        # per-quarter matmuls below must all accumulate
                        nc.tensor.matmul(ps_v, lhsT=or_sb, rhs=bvr_sb,
                                         start=True, stop=False)
                        for t in range(NKT):
                            xt_t = xs.tile([128, CH], F16, name="xt_t")
                            eng = nc.gpsimd if t % 2 == 0 else nc.scalar
                            eng.dma_start(
                                out=xt_t, in_=xt[t * 128:(t + 1) * 128, sq])
                            st, sp = (t == 0), (t == NKT - 1)
                            for h in range(HPG):
                                nc.tensor.matmul(
                                    ps_q[h],
                                    lhsT=wq_sb[:, t, h * D:(h + 1) * D],
                                    rhs=xt_t, start=st, stop=sp)
                            nc.tensor.matmul(ps_k, lhsT=wk_sb[:, t, :], rhs=xt_t,
                                             start=st, stop=sp)
                            # V[s, d] directly: xt slices stationary, wv moving
                            for u in range(HPG):
                                nc.tensor.matmul(
                                    ps_v[:, u * D:(u + 1) * D],
                                    lhsT=xt_t[:, u * 128:(u + 1) * 128],
                                    rhs=wv_sb[:, t, :], start=False, stop=sp)
                        for h in range(HPG):
                            nc.vector.tensor_scalar_add(qt_sb[:, h, sq],
                                                        ps_q[h],
                                                        bq_sb[:, h:h + 1])
                        nc.vector.tensor_scalar_add(kt_sb[:, sq], ps_k, bk_sb)
                        for u in range(HPG):
                            nc.scalar.copy(v_sb[:, 4 * c + u, :],
                                           ps_v[:, u * D:(u + 1) * D])
                        if c == 0:
                            nc.sync.dma_start(
                                out=wo_sb,
                                in_=wo.ap().rearrange("(t p) n -> p t n", p=128))

                # ============ Phase B: attention (+AllGathers), then O-proj ============
                ag_outs = {}
                m_alls = {}

                with (
                    tc.tile_pool(name="mo", bufs=2) as mo,
                    tc.tile_pool(name="ost", bufs=3) as ost,
                ):
                    def attn_chunk(c, ps_s_pool, ps_pv_pool, ps_l_pool, pp,
                                   att, rbp):
                        njt = 4 * c + 4
                        ag_in = dram.tile([HPG, 128, CH], F16, name="ag_in")
                        for h in range(HPG):
                            ps_pv = ps_pv_pool.tile([128, CH], F32, name="ps_pv")
                            ps_l = ps_l_pool.tile([1, CH], F32, name="ps_l")
                            stiles = {}
                            ptiles = {}

                            def emit_s(j):
                                r = max(0, j - 4 * c)
                                cs = slice(128 * r, CH)
                                qs_ = slice(c * CH + 128 * r, (c + 1) * CH)
                                ps_s = ps_s_pool.tile([128, CH], F32,
                                                      name="ps_s")
                                nc.tensor.matmul(
                                    ps_s[:, cs],
                                    lhsT=kt_sb[:, j * 128:(j + 1) * 128],
                                    rhs=qt_sb[:, h, qs_], start=True, stop=True)
                                stiles[j] = (ps_s, cs, r)

                            def emit_p(j):
                                ps_s, cs, r = stiles.pop(j)
                                p_sb = pp.tile([128, CH], F16, name="p_sb")
                                nc.scalar.activation(p_sb[:, cs], ps_s[:, cs],
                                                     Exp, scale=SCALE,
                                                     bias=kb_sb[:, j:j + 1])
                                if j >= 4 * c:
                                    nc.vector.tensor_mul(
                                        p_sb[:, 128 * r:128 * (r + 1)],
                                        p_sb[:, 128 * r:128 * (r + 1)],
                                        mask_sb)
                                ptiles[j] = (p_sb, cs)

                            def emit_pv(j):
                                p_sb, cs = ptiles.pop(j)
                                st, sp = (j == 0), (j == njt - 1)
                                nc.tensor.matmul(ps_pv[:, cs],
                                                 lhsT=v_sb[:, j, :],
                                                 rhs=p_sb[:, cs],
                                                 start=st, stop=sp)
                                nc.tensor.matmul(ps_l[:, cs], lhsT=ones_sb,
                                                 rhs=p_sb[:, cs],
                                                 start=st, stop=sp)

                            emit_s(0)
                            emit_p(0)
                            for j in range(njt):
                                if j + 1 < njt:
                                    emit_s(j + 1)
                                    emit_p(j + 1)
                                emit_pv(j)

                            rl = rbp.tile([1, CH], F32, name="rl")
                            nc.vector.reciprocal_approx_fast(rl, ps_l)
                            rb = rbp.tile([128, CH], F32, name="rb")
                            nc.gpsimd.partition_broadcast(rb, rl, channels=128)
                            at_sb = att.tile([128, CH], F16, name="at_sb")
                            nc.vector.tensor_mul(at_sb, ps_pv, rb)
                            nc.sync.dma_start(out=ag_in[h], in_=at_sb)
                        ag_out = dram.tile([G, HPG, 128, CH], F16,
                                           name="ag_out")
                        nc.gpsimd.collective_compute(
                            "AllGather", mybir.AluOpType.bypass,
                            replica_groups=groups,
                            ins=[ag_in.opt()], outs=[ag_out.opt()],
                        )
                        ag_outs[c] = ag_out

                    def load_mall(c):
                        m_all = mo.tile([128, NKT, CH], F16, name="m_all")
                        ag_r = ag_outs[c].rearrange("g h p n -> p (g h) n")
                        for ct in range(NKT):
                            eng = nc.sync if ct % 2 == 0 else nc.gpsimd
                            eng.dma_start(out=m_all[:, ct, :],
                                          in_=ag_r[:, ct, :])
                        m_alls[c] = m_all

                    def oproj_chunk(c, ps_o_pool):
                        sq = slice(c * CH, (c + 1) * CH)
                        m_all = m_alls.pop(c)
                        for t in range(HPG):
                            ps_o = ps_o_pool.tile([128, CH], F32, name="ps_o")
                            for ct in range(NKT):
                                nc.tensor.matmul(
                                    ps_o,
                                    lhsT=wo_sb[:, ct, t * 128:(t + 1) * 128],
                                    rhs=m_all[:, ct, :],
                                    start=(ct == 0), stop=(ct == NKT - 1))
                            o_sb = ost.tile([128, CH], F16, name="o_sb")
                            nc.vector.tensor_scalar_add(o_sb, ps_o,
                                                        bo_sb[:, t:t + 1])
                            nc.sync.dma_start(
                                out=out[t * 128:(t + 1) * 128, sq], in_=o_sb)

                    # All attention first: every AllGather fires early and
                    # completes while the PE is still busy with later
                    # attention chunks; O-projections run last with their
                    # gathered inputs prefetched one chunk ahead.
                    with (
                        tc.tile_pool(name="ps_s", bufs=3, space="PSUM") as ps_s_pool,
                        tc.tile_pool(name="ps_pv", bufs=3, space="PSUM") as ps_pv_pool,
                        tc.tile_pool(name="ps_l", bufs=2, space="PSUM") as ps_l_pool,
                        tc.tile_pool(name="pp", bufs=4) as pp,
                        tc.tile_pool(name="att", bufs=2) as att,
                        tc.tile_pool(name="rbp", bufs=2) as rbp,
                    ):
                        attn_chunk(0, ps_s_pool, ps_pv_pool, ps_l_pool, pp, att, rbp)
                        attn_chunk(1, ps_s_pool, ps_pv_pool, ps_l_pool, pp, att, rbp)
                        attn_chunk(2, ps_s_pool, ps_pv_pool, ps_l_pool, pp, att, rbp)
                        load_mall(0)
                        attn_chunk(3, ps_s_pool, ps_pv_pool, ps_l_pool, pp, att, rbp)
                        load_mall(1)
                    with tc.tile_pool(name="ps_o", bufs=2, space="PSUM") as ps_o_pool:
                        oproj_chunk(0, ps_o_pool)
                        load_mall(2)
                        oproj_chunk(1, ps_o_pool)
                        load_mall(3)
                        oproj_chunk(2, ps_o_pool)
                        oproj_chunk(3, ps_o_pool)

    nc.compile()
    return nc


def _host_consts():
    p = np.arange(128)
    mask = (p[None, :] >= p[:, None]).astype(np.float16)  # col >= row
    ones = np.ones((128, 1), dtype=np.float16)
    onesrow = np.ones((1, 128), dtype=np.float16)
    return mask, ones, onesrow


def kernel(hidden_states, attention_mask, Wq, bq, Wk, bk, Wv, bv, Wo, bo):
    from concourse.bass_utils import run_bass_kernel_spmd

    global _CACHED_NC
    if _CACHED_NC is None:
        _CACHED_NC = _build_nc()
    nc = _CACHED_NC

    X = np.asarray(hidden_states, dtype=np.float32)
    am = np.asarray(attention_mask).astype(np.float32)  # [B, S] key mask
    Wq = np.asarray(Wq, np.float16)
    Wk = np.asarray(Wk, np.float16)
    Wv = np.asarray(Wv, np.float16)
    Wo = np.asarray(Wo, np.float16)
    mask, ones, onesrow = _host_consts()

    in_maps = []
    for c in range(8):
        b, g = divmod(c, G)
        qs = slice(g * HPG * D, (g + 1) * HPG * D)   # q-head cols of group g
        ks = slice(g * D, (g + 1) * D)               # kv-head cols of group g
        in_maps.append({
            "xt": np.ascontiguousarray(X[b].T.astype(np.float16)),
            "wq": np.ascontiguousarray(Wq[:, qs]),
            "wk": np.ascontiguousarray(Wk[:, ks]),
            "wv": np.ascontiguousarray(Wv[:, ks]),
            "wo": np.ascontiguousarray(Wo[:, qs]),   # hid cols [512g, 512g+512)
            "bq": np.ascontiguousarray(
                np.asarray(bq, np.float32)[qs].reshape(HPG, D).T),
            "bk": np.asarray(bk, np.float32)[ks].reshape(D, 1).copy(),
            "bvr": np.ascontiguousarray(
                np.tile(np.asarray(bv, np.float16)[ks], HPG).reshape(1, HPG * D)),
            "bo": np.ascontiguousarray(
                np.asarray(bo, np.float32)[qs].reshape(HPG, D).T),
            "mask": mask.copy(),
            "keybias": np.ascontiguousarray(
                ((1.0 - am[b]) * -10000.0).astype(np.float32)
                .reshape(NKT, 128).T),
            "ones": ones.copy(),
            "onesrow": onesrow.copy(),
        })

    global _last_in_maps
    _last_in_maps = in_maps
    res = run_bass_kernel_spmd(nc, in_maps, core_ids=list(range(8)))
    out = np.empty((B, S, HID), dtype=np.float32)
    for c in range(8):
        b, g = divmod(c, G)
        out[b][:, g * CH:(g + 1) * CH] = res.results[c]["out"].T.astype(np.float32)
    return out
